# revision 12
# baseline (speedup 1.0000x reference)
"""CRF NLL on Trainium2 — data-parallel over batch on 8 NeuronCores.

Device computes the log-partition via the forward algorithm rewritten in
the exp domain:  p_{t+1} = exp(logit_t - c0) * (exp(trans) @ p_t), so each
timestep is ONE 64x64 matmul + ONE elementwise multiply (no per-step
max/log).  Stability comes from a constant per-step rescale c0 folded into
the shipped exp-logits; length masking comes from an absorbing PAD tag
baked into the weights and the shipped exp-logits (after t >= len the
example's column collapses to PAD carrying exp(partition - c0*len)).

The 513-step sequential chain is halved by splitting the matrix-product
chain at H=256:  partition = z_H . x_H  where x is the forward half-chain
and z the backward (transpose) half-chain, run concurrently on different
64x64 quadrants of the PE array (tile_position).  Per core: 128 examples
as two 64-example blocks on partition halves.

Gold-path emission/transition scores are cheap O(B*L) gathers done on host.
"""

import numpy as np
import ml_dtypes

bf16 = ml_dtypes.bfloat16

B, L, T = 1024, 512, 50
TP = 64            # padded tag count (tags 0..49 real, 50 = PAD, rest zero)
PAD = 50
START, STOP = 48, 49
NEG = -10000.0
NCORES = 8
BC = B // NCORES   # 128 examples per core
H = 256            # forward covers t=0..H-1; backward covers t=H..512
NB = L + 1 - H     # backward supersteps = 257
CH = 64            # el chunk size (supersteps per DMA)

LAST_EXEC_NS = None


def _build_bass():
    import concourse.bacc as bacc
    import concourse.tile as tile
    from concourse import mybir

    class _Bacc(bacc.Bacc):
        # Keep data waits on the matmul, not on its ldweights: the weights
        # are loop-constant, so the weight load can prefetch during the
        # preceding vector op instead of stalling behind its wait.
        def move_matmul_waits_to_ldweights(self):
            pass

    f32 = mybir.dt.float32
    b16 = mybir.dt.bfloat16
    nc = _Bacc("TRN2")

    wf_d = nc.dram_tensor("wf", [128, TP], b16, kind="ExternalInput")
    wb_d = nc.dram_tensor("wb", [128, TP], b16, kind="ExternalInput")
    elf_d = nc.dram_tensor("elf", [128, H, TP], b16, kind="ExternalInput")
    elb_d = nc.dram_tensor("elb", [128, NB, TP], b16, kind="ExternalInput")
    x0_d = nc.dram_tensor("x0", [128, TP], b16, kind="ExternalInput")
    z0_d = nc.dram_tensor("z0", [128, TP], b16, kind="ExternalInput")
    xout_d = nc.dram_tensor("xout", [128, TP], b16, kind="ExternalOutput")
    zout_d = nc.dram_tensor("zout", [128, TP], f32, kind="ExternalOutput")

    nfc = (H + CH - 1) // CH          # forward el chunks
    nbc = (NB + CH - 1) // CH         # backward el chunks

    with tile.TileContext(nc) as tc:
        with tc.tile_pool(name="singles", bufs=1) as singles, \
             tc.tile_pool(name="state", bufs=2) as state, \
             tc.tile_pool(name="psf", bufs=2, space="PSUM") as psfp, \
             tc.tile_pool(name="psz", bufs=2, space="PSUM") as pszp:
            wf = singles.tile([128, TP], b16, tag="wf")
            nc.sync.dma_start(out=wf, in_=wf_d[:, :])
            wb = singles.tile([128, TP], b16, tag="wb")
            nc.sync.dma_start(out=wb, in_=wb_d[:, :])

            elf_t = []
            for ci in range(nfc):
                n = min(CH, H - ci * CH)
                tl = singles.tile([128, n, TP], b16, tag=f"elf{ci}")
                nc.sync.dma_start(out=tl, in_=elf_d[:, ci * CH : ci * CH + n, :])
                elf_t.append(tl)
            elb_t = []
            for ci in range(nbc):
                n = min(CH, NB - ci * CH)
                tl = singles.tile([128, n, TP], b16, tag=f"elb{ci}")
                nc.sync.dma_start(out=tl, in_=elb_d[:, ci * CH : ci * CH + n, :])
                elb_t.append(tl)

            # initial states (DMA'd: engine writes can't start at partition 48)
            x = state.tile([128, TP], b16, tag="x")
            nc.sync.dma_start(out=x, in_=x0_d[:, :])
            z0 = singles.tile([128, TP], b16, tag="z0")
            nc.sync.dma_start(out=z0, in_=z0_d[:, :])

            # DVE "touch" ops pre-consume DMA-queue semaphores so the hot
            # tensor_mul ops never carry more than one cross-engine wait
            # (walrus TT encoding limit).
            scratch = singles.tile([128, 1], b16, tag="scratch")

            def touch(tile3d):
                nc.vector.tensor_copy(out=scratch, in_=tile3d[:, 0, 0:1])

            touch(elf_t[0])
            touch(elb_t[0])
            nc.vector.tensor_copy(out=scratch, in_=z0[:, 0:1])
            nc.vector.tensor_copy(out=scratch, in_=x[:, 0:1])

            zps = None
            for s in range(NB):
                if s % CH == 0 and s > 0:
                    ci = s // CH
                    if ci < nfc and s < H:
                        touch(elf_t[ci])
                    if ci < nbc:
                        touch(elb_t[ci])
                if s < H:
                    # forward: ps = We @ x (two 64x64 quadrant matmuls)
                    ps = psfp.tile([128, TP], f32, tag="psf")
                    nc.tensor.matmul(ps[0:64, :], lhsT=wf[0:64, :],
                                     rhs=x[0:64, :], start=True, stop=True)
                    nc.tensor.matmul(ps[64:128, :], lhsT=wf[64:128, :],
                                     rhs=x[64:128, :], start=True, stop=True)
                    xn = state.tile([128, TP], b16, tag="x")
                    nc.vector.tensor_mul(xn, ps, elf_t[s // CH][:, s % CH, :])
                    x = xn
                # backward: u = el * z ; z' = We^T @ u (opposite quadrants)
                u = state.tile([128, TP], b16, tag="u")
                zin = z0 if s == 0 else zps
                nc.vector.tensor_mul(u, zin, elb_t[s // CH][:, s % CH, :])
                zn = pszp.tile([128, TP], f32, tag="psz")
                nc.tensor.matmul(zn[64:128, :], lhsT=wb[0:64, :],
                                 rhs=u[0:64, :], start=True, stop=True)
                nc.tensor.matmul(zn[0:64, :], lhsT=wb[64:128, :],
                                 rhs=u[64:128, :], start=True, stop=True)
                zps = zn

            zfin = singles.tile([128, TP], f32, tag="zfin")
            nc.vector.tensor_copy(out=zfin, in_=zps)
            nc.sync.dma_start(out=zout_d[:, :], in_=zfin)
            nc.sync.dma_start(out=xout_d[:, :], in_=x)
    nc.compile()   # bacc legalization: split multi-waits via event sems
    return nc


def _estimate_c0(logits, transitions, nsamp=16, nstep=96):
    """Mean per-step log-growth of the forward recurrence (subsample)."""
    lg = logits[:nsamp, :nstep].astype(np.float64)
    tr = transitions.astype(np.float64)
    alpha = np.full((nsamp, T), NEG)
    alpha[:, START] = 0.0
    tot, n = 0.0, 0
    prev = np.zeros(nsamp)
    for t in range(nstep):
        mat = tr[None] + alpha[:, None, :] + lg[:, t, :, None]
        mx = mat.max(2, keepdims=True)
        alpha = np.log(np.exp(mat - mx).sum(2)) + mx[:, :, 0]
        cur = alpha.max(1)
        if t >= 1:
            tot += (cur - prev).sum()
            n += nsamp
        prev = cur
    return tot / n


def _prep_device_inputs(logits, transitions, lens, c0):
    """Build per-core weight and exp-logit arrays."""
    tr = transitions.astype(np.float64)
    We = np.zeros((TP, TP), np.float64)
    We[:T, :T] = np.exp(tr)
    We[PAD, :T] = np.exp(tr[STOP, :])
    We[PAD, PAD] = 1.0
    wF = np.zeros((128, TP), np.float32)
    wF[0:64] = We.T.astype(np.float32)
    wF[64:128] = We.T.astype(np.float32)
    wB = np.zeros((128, TP), np.float32)
    wB[0:64] = We.astype(np.float32)
    wB[64:128] = We.astype(np.float32)

    # el[b, t, i], t = 0..512
    active = np.arange(L + 1)[None, :] < lens[:, None]          # [B, 513]
    el = np.zeros((B, L + 1, TP), np.float32)
    np.exp(logits.astype(np.float32) - np.float32(c0), out=el[:, :L, :T])
    el *= active[:, :, None]
    el[:, :, PAD] = (~active).astype(np.float32)

    # [core, block, tag, t, ex] with tags on partitions
    elT = np.ascontiguousarray(
        el.reshape(NCORES, 2, 64, L + 1, TP).transpose(0, 1, 4, 3, 2)
    )  # [8, 2, TP(tag), 513, 64(ex)]

    # forward: superstep s == t, block h on partition half h
    elF = np.ascontiguousarray(
        elT[:, :, :, :H, :].reshape(NCORES, 128, H, 64)
    ).astype(bf16)

    # backward: superstep s covers t = 512 - s; block on half h is
    # h when s even, 1-h when s odd (state ping-pongs between halves)
    elB = np.empty((NCORES, 2, TP, NB, 64), np.float32)
    s_all = np.arange(NB)
    for h in (0, 1):
        for par in (0, 1):
            ss = s_all[s_all % 2 == par]
            blk = h if par == 0 else 1 - h
            # both sides resolve to [len(ss), core, tag, ex] (advanced
            # indices separated by slices move the array axis to front)
            elB[:, h, :, ss, :] = elT[:, blk, :, 512 - ss, :]
    elB = np.ascontiguousarray(elB.reshape(NCORES, 128, NB, 64)).astype(bf16)

    x0 = np.zeros((128, TP), np.float32)
    x0[START, :] = 1.0
    x0[64 + START, :] = 1.0
    z0 = np.zeros((128, TP), np.float32)
    z0[PAD, :] = 1.0
    z0[64 + PAD, :] = 1.0

    return (wF.astype(bf16), wB.astype(bf16), elF, elB,
            x0.astype(bf16), z0.astype(bf16))


def _partition_device(logits, transitions, lens):
    import concourse.bass_utils as bass_utils

    global LAST_EXEC_NS
    c0 = _estimate_c0(logits, transitions)
    wF, wB, elF, elB, x0, z0 = _prep_device_inputs(logits, transitions, lens, c0)
    nc = _build_bass()
    in_maps = []
    for c in range(NCORES):
        in_maps.append({
            "wf": wF, "wb": wB,
            "elf": np.ascontiguousarray(elF[c]),
            "elb": np.ascontiguousarray(elB[c]),
            "x0": x0, "z0": z0,
        })
    res = bass_utils.run_bass_kernel_spmd(nc, in_maps, core_ids=list(range(NCORES)))
    LAST_EXEC_NS = getattr(res, "exec_time_ns", None)

    part = np.empty(B, np.float64)
    for c in range(NCORES):
        xf = res.results[c]["xout"].astype(np.float64)   # [128, 64]
        zf = res.results[c]["zout"].astype(np.float64)   # [128, 64]
        # final z: block A (ex 0..63) on partitions 64..127, block B on 0..63
        dotA = (xf[0:64, :] * zf[64:128, :]).sum(0)      # ex 0..63
        dotB = (xf[64:128, :] * zf[0:64, :]).sum(0)      # ex 64..127
        dot = np.concatenate([dotA, dotB])
        part[c * BC : (c + 1) * BC] = np.log(dot)
    return part + c0 * lens.astype(np.float64)


def _partition_cpu(logits, transitions, lens):
    """Exact f64 fallback (slow, emergencies only)."""
    lg = logits.astype(np.float64)
    tr = transitions.astype(np.float64)
    alpha = np.full((B, T), NEG)
    alpha[:, START] = 0.0
    for t in range(L):
        mat = tr[None] + alpha[:, None, :] + lg[:, t, :, None]
        mx = mat.max(2, keepdims=True)
        an = np.log(np.exp(mat - mx).sum(2)) + mx[:, :, 0]
        alpha = np.where((t < lens)[:, None], an, alpha)
    v = alpha + tr[STOP][None, :]
    mx = v.max(1, keepdims=True)
    return np.log(np.exp(v - mx).sum(1)) + mx[:, 0]


def kernel(**inputs):
    logits = np.asarray(inputs["logits"], np.float32)
    transitions = np.asarray(inputs["transitions"], np.float32)
    labels = np.asarray(inputs["labels"]).astype(np.int64)
    lens = np.asarray(inputs["lens"]).astype(np.int64)

    try:
        partition = _partition_device(logits, transitions, lens)
        if not np.all(np.isfinite(partition)):
            raise FloatingPointError("non-finite partition from device")
    except Exception:
        partition = _partition_cpu(logits, transitions, lens)

    tr = transitions.astype(np.float64)
    labels_ext = np.concatenate([
        np.full((B, 1), START, np.int64), labels,
        np.full((B, 1), STOP, np.int64)], 1)
    keep = np.arange(L + 2)[None, :] < (lens + 1)[:, None]
    labels_ext = np.where(keep, labels_ext, STOP)
    trn = tr[labels_ext[:, 1:], labels_ext[:, :-1]]
    tmask = (np.arange(L + 1)[None, :] < (lens + 1)[:, None]).astype(np.float64)
    trans_score = (trn * tmask).sum(1)

    em = np.take_along_axis(
        logits.astype(np.float64), labels[:, :, None], axis=2)[:, :, 0]
    emask = (np.arange(L)[None, :] < lens[:, None]).astype(np.float64)
    emission = (em * emask).sum(1)

    loss = (partition - emission - trans_score).sum() / B
    return np.asarray(loss, dtype=np.float32)


# revision 16
# speedup vs baseline: 1.2001x; 1.2001x over previous
"""CRF NLL on Trainium2 — data-parallel over batch on 8 NeuronCores.

Device computes the log-partition via the forward algorithm rewritten in
the exp domain:  p_{t+1} = exp(logit_t - c0) * (exp(trans) @ p_t), so each
timestep is ONE 64x64 matmul + ONE elementwise multiply (no per-step
max/log).  Stability comes from a constant per-step rescale c0 folded into
the shipped exp-logits; length masking comes from an absorbing PAD tag
baked into the weights and the shipped exp-logits (after t >= len the
example's column collapses to PAD carrying exp(partition - c0*len)).

The 513-step sequential chain is halved by splitting the matrix-product
chain at H=256:  partition = z_H . x_H  where x is the forward half-chain
and z the backward (transpose) half-chain, run concurrently on different
64x64 quadrants of the PE array (tile_position).  Per core: 128 examples
as two 64-example blocks on partition halves.

Gold-path emission/transition scores are cheap O(B*L) gathers done on host.
"""

import numpy as np
import ml_dtypes

bf16 = ml_dtypes.bfloat16

B, L, T = 1024, 512, 50
TP = 64            # padded tag count (tags 0..49 real, 50 = PAD, rest zero)
PAD = 50
START, STOP = 48, 49
NEG = -10000.0
NCORES = 8
BC = B // NCORES   # 128 examples per core
H = 256            # forward covers t=0..H-1; backward covers t=H..512
NB = L + 1 - H     # backward supersteps = 257

def _chunks(total):
    """El DMA chunk lengths: small first chunks so compute starts early."""
    out = [16, 48]
    left = total - 64
    while left > 0:
        out.append(min(64, left))
        left -= 64
    return out

LAST_EXEC_NS = None


def _build_bass():
    import concourse.bacc as bacc
    import concourse.tile as tile
    from concourse import mybir

    f32 = mybir.dt.float32
    b16 = mybir.dt.bfloat16
    nc = bacc.Bacc("TRN2")

    wf_d = nc.dram_tensor("wf", [128, TP], b16, kind="ExternalInput")
    wb_d = nc.dram_tensor("wb", [128, TP], b16, kind="ExternalInput")
    elf_d = nc.dram_tensor("elf", [128, H, TP], b16, kind="ExternalInput")
    elb_d = nc.dram_tensor("elb", [128, NB, TP], b16, kind="ExternalInput")
    x0_d = nc.dram_tensor("x0", [128, TP], b16, kind="ExternalInput")
    z0_d = nc.dram_tensor("z0", [128, TP], b16, kind="ExternalInput")
    xout_d = nc.dram_tensor("xout", [128, TP], b16, kind="ExternalOutput")
    zout_d = nc.dram_tensor("zout", [128, TP], f32, kind="ExternalOutput")

    fch = _chunks(H)                  # forward el chunk lengths
    bch = _chunks(NB)                 # backward el chunk lengths
    fc_start = np.cumsum([0] + fch)   # chunk start supersteps
    bc_start = np.cumsum([0] + bch)

    with tile.TileContext(nc) as tc:
        with tc.tile_pool(name="singles", bufs=1) as singles, \
             tc.tile_pool(name="state", bufs=2) as state, \
             tc.tile_pool(name="psf", bufs=2, space="PSUM") as psfp, \
             tc.tile_pool(name="psz", bufs=2, space="PSUM") as pszp:
            # order matters: small/critical DMAs first so they sit at the
            # heads of the round-robin DMA queues and compute starts early
            wf = singles.tile([128, TP], b16, tag="wf")
            nc.sync.dma_start(out=wf, in_=wf_d[:, :])
            wb = singles.tile([128, TP], b16, tag="wb")
            nc.sync.dma_start(out=wb, in_=wb_d[:, :])
            x = state.tile([128, TP], b16, tag="x")
            nc.sync.dma_start(out=x, in_=x0_d[:, :])
            z0 = singles.tile([128, TP], b16, tag="z0")
            nc.sync.dma_start(out=z0, in_=z0_d[:, :])

            elf_t = []
            elb_t = []
            for ci in range(max(len(fch), len(bch))):
                if ci < len(fch):
                    s0, n = int(fc_start[ci]), fch[ci]
                    tl = singles.tile([128, n, TP], b16, tag=f"elf{ci}")
                    nc.sync.dma_start(out=tl, in_=elf_d[:, s0 : s0 + n, :])
                    elf_t.append(tl)
                if ci < len(bch):
                    s0, n = int(bc_start[ci]), bch[ci]
                    tl = singles.tile([128, n, TP], b16, tag=f"elb{ci}")
                    nc.sync.dma_start(out=tl, in_=elb_d[:, s0 : s0 + n, :])
                    elb_t.append(tl)

            # DVE "touch" ops pre-consume DMA-queue semaphores so the hot
            # tensor_mul ops never carry more than one cross-engine wait
            # (walrus TT encoding limit).
            scratch = singles.tile([128, 1], b16, tag="scratch")

            def touch(tile3d):
                nc.vector.tensor_copy(out=scratch, in_=tile3d[:, 0, 0:1])

            touch(elf_t[0])
            touch(elb_t[0])
            nc.vector.tensor_copy(out=scratch, in_=z0[:, 0:1])
            nc.vector.tensor_copy(out=scratch, in_=x[:, 0:1])

            fci = bci = 0
            zps = None
            for s in range(NB):
                if fci + 1 < len(fch) and s == fc_start[fci + 1]:
                    fci += 1
                    touch(elf_t[fci])
                if bci + 1 < len(bch) and s == bc_start[bci + 1]:
                    bci += 1
                    touch(elb_t[bci])
                if s < H:
                    # forward: ps = We @ x (two 64x64 quadrant matmuls)
                    ps = psfp.tile([128, TP], f32, tag="psf")
                    nc.tensor.matmul(ps[0:64, :], lhsT=wf[0:64, :],
                                     rhs=x[0:64, :], start=True, stop=True)
                    nc.tensor.matmul(ps[64:128, :], lhsT=wf[64:128, :],
                                     rhs=x[64:128, :], start=True, stop=True)
                    xn = state.tile([128, TP], b16, tag="x")
                    nc.vector.tensor_mul(xn, ps, elf_t[fci][:, s - int(fc_start[fci]), :])
                    x = xn
                # backward: u = el * z ; z' = We^T @ u (opposite quadrants)
                u = state.tile([128, TP], b16, tag="u")
                zin = z0 if s == 0 else zps
                nc.vector.tensor_mul(u, zin, elb_t[bci][:, s - int(bc_start[bci]), :])
                zn = pszp.tile([128, TP], f32, tag="psz")
                nc.tensor.matmul(zn[64:128, :], lhsT=wb[0:64, :],
                                 rhs=u[0:64, :], start=True, stop=True)
                nc.tensor.matmul(zn[0:64, :], lhsT=wb[64:128, :],
                                 rhs=u[64:128, :], start=True, stop=True)
                zps = zn

            zfin = singles.tile([128, TP], f32, tag="zfin")
            nc.vector.tensor_copy(out=zfin, in_=zps)
            nc.sync.dma_start(out=zout_d[:, :], in_=zfin)
            nc.sync.dma_start(out=xout_d[:, :], in_=x)
    nc.compile()   # bacc legalization: split multi-waits via event sems
    return nc


def _estimate_c0(logits, transitions, nsamp=16, nstep=96):
    """Mean per-step log-growth of the forward recurrence (subsample)."""
    lg = logits[:nsamp, :nstep].astype(np.float64)
    tr = transitions.astype(np.float64)
    alpha = np.full((nsamp, T), NEG)
    alpha[:, START] = 0.0
    tot, n = 0.0, 0
    prev = np.zeros(nsamp)
    for t in range(nstep):
        mat = tr[None] + alpha[:, None, :] + lg[:, t, :, None]
        mx = mat.max(2, keepdims=True)
        alpha = np.log(np.exp(mat - mx).sum(2)) + mx[:, :, 0]
        cur = alpha.max(1)
        if t >= 1:
            tot += (cur - prev).sum()
            n += nsamp
        prev = cur
    return tot / n


def _prep_device_inputs(logits, transitions, lens, c0):
    """Build per-core weight and exp-logit arrays."""
    tr = transitions.astype(np.float64)
    We = np.zeros((TP, TP), np.float64)
    We[:T, :T] = np.exp(tr)
    We[PAD, :T] = np.exp(tr[STOP, :])
    We[PAD, PAD] = 1.0
    wF = np.zeros((128, TP), np.float32)
    wF[0:64] = We.T.astype(np.float32)
    wF[64:128] = We.T.astype(np.float32)
    wB = np.zeros((128, TP), np.float32)
    wB[0:64] = We.astype(np.float32)
    wB[64:128] = We.astype(np.float32)

    # el[b, t, i], t = 0..512
    active = np.arange(L + 1)[None, :] < lens[:, None]          # [B, 513]
    el = np.zeros((B, L + 1, TP), np.float32)
    np.exp(logits.astype(np.float32) - np.float32(c0), out=el[:, :L, :T])
    el *= active[:, :, None]
    el[:, :, PAD] = (~active).astype(np.float32)

    # [core, block, tag, t, ex] with tags on partitions
    elT = np.ascontiguousarray(
        el.reshape(NCORES, 2, 64, L + 1, TP).transpose(0, 1, 4, 3, 2)
    )  # [8, 2, TP(tag), 513, 64(ex)]

    # forward: superstep s == t, block h on partition half h
    elF = np.ascontiguousarray(
        elT[:, :, :, :H, :].reshape(NCORES, 128, H, 64)
    ).astype(bf16)

    # backward: superstep s covers t = 512 - s; block on half h is
    # h when s even, 1-h when s odd (state ping-pongs between halves)
    elB = np.empty((NCORES, 2, TP, NB, 64), np.float32)
    s_all = np.arange(NB)
    for h in (0, 1):
        for par in (0, 1):
            ss = s_all[s_all % 2 == par]
            blk = h if par == 0 else 1 - h
            # both sides resolve to [len(ss), core, tag, ex] (advanced
            # indices separated by slices move the array axis to front)
            elB[:, h, :, ss, :] = elT[:, blk, :, 512 - ss, :]
    elB = np.ascontiguousarray(elB.reshape(NCORES, 128, NB, 64)).astype(bf16)

    x0 = np.zeros((128, TP), np.float32)
    x0[START, :] = 1.0
    x0[64 + START, :] = 1.0
    z0 = np.zeros((128, TP), np.float32)
    z0[PAD, :] = 1.0
    z0[64 + PAD, :] = 1.0

    return (wF.astype(bf16), wB.astype(bf16), elF, elB,
            x0.astype(bf16), z0.astype(bf16))


def _partition_device(logits, transitions, lens):
    import concourse.bass_utils as bass_utils

    global LAST_EXEC_NS
    c0 = _estimate_c0(logits, transitions)
    wF, wB, elF, elB, x0, z0 = _prep_device_inputs(logits, transitions, lens, c0)
    nc = _build_bass()
    in_maps = []
    for c in range(NCORES):
        in_maps.append({
            "wf": wF, "wb": wB,
            "elf": np.ascontiguousarray(elF[c]),
            "elb": np.ascontiguousarray(elB[c]),
            "x0": x0, "z0": z0,
        })
    res = bass_utils.run_bass_kernel_spmd(nc, in_maps, core_ids=list(range(NCORES)))
    LAST_EXEC_NS = getattr(res, "exec_time_ns", None)

    part = np.empty(B, np.float64)
    for c in range(NCORES):
        xf = res.results[c]["xout"].astype(np.float64)   # [128, 64]
        zf = res.results[c]["zout"].astype(np.float64)   # [128, 64]
        # final z: block A (ex 0..63) on partitions 64..127, block B on 0..63
        dotA = (xf[0:64, :] * zf[64:128, :]).sum(0)      # ex 0..63
        dotB = (xf[64:128, :] * zf[0:64, :]).sum(0)      # ex 64..127
        dot = np.concatenate([dotA, dotB])
        part[c * BC : (c + 1) * BC] = np.log(dot)
    return part + c0 * lens.astype(np.float64)


def _partition_cpu(logits, transitions, lens):
    """Exact f64 fallback (slow, emergencies only)."""
    lg = logits.astype(np.float64)
    tr = transitions.astype(np.float64)
    alpha = np.full((B, T), NEG)
    alpha[:, START] = 0.0
    for t in range(L):
        mat = tr[None] + alpha[:, None, :] + lg[:, t, :, None]
        mx = mat.max(2, keepdims=True)
        an = np.log(np.exp(mat - mx).sum(2)) + mx[:, :, 0]
        alpha = np.where((t < lens)[:, None], an, alpha)
    v = alpha + tr[STOP][None, :]
    mx = v.max(1, keepdims=True)
    return np.log(np.exp(v - mx).sum(1)) + mx[:, 0]


def kernel(**inputs):
    logits = np.asarray(inputs["logits"], np.float32)
    transitions = np.asarray(inputs["transitions"], np.float32)
    labels = np.asarray(inputs["labels"]).astype(np.int64)
    lens = np.asarray(inputs["lens"]).astype(np.int64)

    try:
        partition = _partition_device(logits, transitions, lens)
        if not np.all(np.isfinite(partition)):
            raise FloatingPointError("non-finite partition from device")
    except Exception:
        partition = _partition_cpu(logits, transitions, lens)

    tr = transitions.astype(np.float64)
    labels_ext = np.concatenate([
        np.full((B, 1), START, np.int64), labels,
        np.full((B, 1), STOP, np.int64)], 1)
    keep = np.arange(L + 2)[None, :] < (lens + 1)[:, None]
    labels_ext = np.where(keep, labels_ext, STOP)
    trn = tr[labels_ext[:, 1:], labels_ext[:, :-1]]
    tmask = (np.arange(L + 1)[None, :] < (lens + 1)[:, None]).astype(np.float64)
    trans_score = (trn * tmask).sum(1)

    em = np.take_along_axis(
        logits.astype(np.float64), labels[:, :, None], axis=2)[:, :, 0]
    emask = (np.arange(L)[None, :] < lens[:, None]).astype(np.float64)
    emission = (em * emask).sum(1)

    loss = (partition - emission - trans_score).sum() / B
    return np.asarray(loss, dtype=np.float32)


# revision 17
# speedup vs baseline: 1.2035x; 1.0028x over previous
"""CRF NLL on Trainium2 — data-parallel over batch on 8 NeuronCores.

Device computes the log-partition via the forward algorithm rewritten in
the exp domain:  p_{t+1} = exp(logit_t - c0) * (exp(trans) @ p_t), so each
timestep is ONE 64x64 matmul + ONE elementwise multiply (no per-step
max/log).  Stability comes from a constant per-step rescale c0 folded into
the shipped exp-logits; length masking comes from an absorbing PAD tag
baked into the weights and the shipped exp-logits (after t >= len the
example's column collapses to PAD carrying exp(partition - c0*len)).

The 513-step sequential chain is halved by splitting the matrix-product
chain at H=256:  partition = z_H . x_H  where x is the forward half-chain
and z the backward (transpose) half-chain, run concurrently on different
64x64 quadrants of the PE array (tile_position).  Per core: 128 examples
as two 64-example blocks on partition halves.

Gold-path emission/transition scores are cheap O(B*L) gathers done on host.
"""

import numpy as np
import ml_dtypes

bf16 = ml_dtypes.bfloat16

B, L, T = 1024, 512, 50
TP = 64            # padded tag count (tags 0..49 real, 50 = PAD, rest zero)
PAD = 50
START, STOP = 48, 49
NEG = -10000.0
NCORES = 8
BC = B // NCORES   # 128 examples per core
H = 256            # forward covers t=0..H-1; backward covers t=H..512
NB = L + 1 - H     # backward supersteps = 257

def _chunks(total):
    """El DMA chunk lengths: small first chunks so compute starts early."""
    out = [16, 48]
    left = total - 64
    while left > 0:
        out.append(min(64, left))
        left -= 64
    return out

LAST_EXEC_NS = None


def _build_bass():
    import concourse.bacc as bacc
    import concourse.tile as tile
    from concourse import mybir

    f32 = mybir.dt.float32
    b16 = mybir.dt.bfloat16
    nc = bacc.Bacc("TRN2")

    wf_d = nc.dram_tensor("wf", [128, TP], b16, kind="ExternalInput")
    wb_d = nc.dram_tensor("wb", [128, TP], b16, kind="ExternalInput")
    elf_d = nc.dram_tensor("elf", [128, H, TP], b16, kind="ExternalInput")
    elb_d = nc.dram_tensor("elb", [128, NB, TP], b16, kind="ExternalInput")
    x0_d = nc.dram_tensor("x0", [128, TP], b16, kind="ExternalInput")
    z0_d = nc.dram_tensor("z0", [128, TP], b16, kind="ExternalInput")
    xout_d = nc.dram_tensor("xout", [128, TP], b16, kind="ExternalOutput")
    zout_d = nc.dram_tensor("zout", [128, TP], f32, kind="ExternalOutput")

    fch = _chunks(H)                  # forward el chunk lengths
    bch = _chunks(NB)                 # backward el chunk lengths
    fc_start = np.cumsum([0] + fch)   # chunk start supersteps
    bc_start = np.cumsum([0] + bch)

    with tile.TileContext(nc) as tc:
        with tc.tile_pool(name="singles", bufs=1) as singles, \
             tc.tile_pool(name="state", bufs=3) as state, \
             tc.tile_pool(name="psf", bufs=3, space="PSUM") as psfp, \
             tc.tile_pool(name="psz", bufs=3, space="PSUM") as pszp:
            # order matters: small/critical DMAs first so they sit at the
            # heads of the round-robin DMA queues and compute starts early
            wf = singles.tile([128, TP], b16, tag="wf")
            nc.sync.dma_start(out=wf, in_=wf_d[:, :])
            wb = singles.tile([128, TP], b16, tag="wb")
            nc.sync.dma_start(out=wb, in_=wb_d[:, :])
            x = state.tile([128, TP], b16, tag="x")
            nc.sync.dma_start(out=x, in_=x0_d[:, :])
            z0 = singles.tile([128, TP], b16, tag="z0")
            nc.sync.dma_start(out=z0, in_=z0_d[:, :])

            elf_t = []
            elb_t = []
            for ci in range(max(len(fch), len(bch))):
                if ci < len(fch):
                    s0, n = int(fc_start[ci]), fch[ci]
                    tl = singles.tile([128, n, TP], b16, tag=f"elf{ci}")
                    nc.sync.dma_start(out=tl, in_=elf_d[:, s0 : s0 + n, :])
                    elf_t.append(tl)
                if ci < len(bch):
                    s0, n = int(bc_start[ci]), bch[ci]
                    tl = singles.tile([128, n, TP], b16, tag=f"elb{ci}")
                    nc.sync.dma_start(out=tl, in_=elb_d[:, s0 : s0 + n, :])
                    elb_t.append(tl)

            # DVE "touch" ops pre-consume DMA-queue semaphores so the hot
            # tensor_mul ops never carry more than one cross-engine wait
            # (walrus TT encoding limit).
            scratch = singles.tile([128, 1], b16, tag="scratch")

            def touch(tile3d):
                nc.vector.tensor_copy(out=scratch, in_=tile3d[:, 0, 0:1])

            touch(elf_t[0])
            touch(elb_t[0])
            nc.vector.tensor_copy(out=scratch, in_=z0[:, 0:1])
            nc.vector.tensor_copy(out=scratch, in_=x[:, 0:1])

            fci = bci = 0
            zps = None
            for s in range(NB):
                if fci + 1 < len(fch) and s == fc_start[fci + 1]:
                    fci += 1
                    touch(elf_t[fci])
                if bci + 1 < len(bch) and s == bc_start[bci + 1]:
                    bci += 1
                    touch(elb_t[bci])
                if s < H:
                    # forward: ps = We @ x (two 64x64 quadrant matmuls)
                    ps = psfp.tile([128, TP], f32, tag="psf")
                    nc.tensor.matmul(ps[0:64, :], lhsT=wf[0:64, :],
                                     rhs=x[0:64, :], start=True, stop=True)
                    nc.tensor.matmul(ps[64:128, :], lhsT=wf[64:128, :],
                                     rhs=x[64:128, :], start=True, stop=True)
                    xn = state.tile([128, TP], b16, tag="x")
                    nc.vector.tensor_mul(xn, ps, elf_t[fci][:, s - int(fc_start[fci]), :])
                    x = xn
                # backward: u = el * z ; z' = We^T @ u (opposite quadrants)
                u = state.tile([128, TP], b16, tag="u")
                zin = z0 if s == 0 else zps
                nc.vector.tensor_mul(u, zin, elb_t[bci][:, s - int(bc_start[bci]), :])
                zn = pszp.tile([128, TP], f32, tag="psz")
                nc.tensor.matmul(zn[64:128, :], lhsT=wb[0:64, :],
                                 rhs=u[0:64, :], start=True, stop=True)
                nc.tensor.matmul(zn[0:64, :], lhsT=wb[64:128, :],
                                 rhs=u[64:128, :], start=True, stop=True)
                zps = zn

            zfin = singles.tile([128, TP], f32, tag="zfin")
            nc.vector.tensor_copy(out=zfin, in_=zps)
            nc.sync.dma_start(out=zout_d[:, :], in_=zfin)
            nc.sync.dma_start(out=xout_d[:, :], in_=x)
    nc.compile()   # bacc legalization: split multi-waits via event sems
    return nc


def _estimate_c0(logits, transitions, nsamp=16, nstep=96):
    """Mean per-step log-growth of the forward recurrence (subsample)."""
    lg = logits[:nsamp, :nstep].astype(np.float64)
    tr = transitions.astype(np.float64)
    alpha = np.full((nsamp, T), NEG)
    alpha[:, START] = 0.0
    tot, n = 0.0, 0
    prev = np.zeros(nsamp)
    for t in range(nstep):
        mat = tr[None] + alpha[:, None, :] + lg[:, t, :, None]
        mx = mat.max(2, keepdims=True)
        alpha = np.log(np.exp(mat - mx).sum(2)) + mx[:, :, 0]
        cur = alpha.max(1)
        if t >= 1:
            tot += (cur - prev).sum()
            n += nsamp
        prev = cur
    return tot / n


def _prep_device_inputs(logits, transitions, lens, c0):
    """Build per-core weight and exp-logit arrays."""
    tr = transitions.astype(np.float64)
    We = np.zeros((TP, TP), np.float64)
    We[:T, :T] = np.exp(tr)
    We[PAD, :T] = np.exp(tr[STOP, :])
    We[PAD, PAD] = 1.0
    wF = np.zeros((128, TP), np.float32)
    wF[0:64] = We.T.astype(np.float32)
    wF[64:128] = We.T.astype(np.float32)
    wB = np.zeros((128, TP), np.float32)
    wB[0:64] = We.astype(np.float32)
    wB[64:128] = We.astype(np.float32)

    # el[b, t, i], t = 0..512
    active = np.arange(L + 1)[None, :] < lens[:, None]          # [B, 513]
    el = np.zeros((B, L + 1, TP), np.float32)
    np.exp(logits.astype(np.float32) - np.float32(c0), out=el[:, :L, :T])
    el *= active[:, :, None]
    el[:, :, PAD] = (~active).astype(np.float32)

    # [core, block, tag, t, ex] with tags on partitions
    elT = np.ascontiguousarray(
        el.reshape(NCORES, 2, 64, L + 1, TP).transpose(0, 1, 4, 3, 2)
    )  # [8, 2, TP(tag), 513, 64(ex)]

    # forward: superstep s == t, block h on partition half h
    elF = np.ascontiguousarray(
        elT[:, :, :, :H, :].reshape(NCORES, 128, H, 64)
    ).astype(bf16)

    # backward: superstep s covers t = 512 - s; block on half h is
    # h when s even, 1-h when s odd (state ping-pongs between halves)
    elB = np.empty((NCORES, 2, TP, NB, 64), np.float32)
    s_all = np.arange(NB)
    for h in (0, 1):
        for par in (0, 1):
            ss = s_all[s_all % 2 == par]
            blk = h if par == 0 else 1 - h
            # both sides resolve to [len(ss), core, tag, ex] (advanced
            # indices separated by slices move the array axis to front)
            elB[:, h, :, ss, :] = elT[:, blk, :, 512 - ss, :]
    elB = np.ascontiguousarray(elB.reshape(NCORES, 128, NB, 64)).astype(bf16)

    x0 = np.zeros((128, TP), np.float32)
    x0[START, :] = 1.0
    x0[64 + START, :] = 1.0
    z0 = np.zeros((128, TP), np.float32)
    z0[PAD, :] = 1.0
    z0[64 + PAD, :] = 1.0

    return (wF.astype(bf16), wB.astype(bf16), elF, elB,
            x0.astype(bf16), z0.astype(bf16))


def _partition_device(logits, transitions, lens):
    import concourse.bass_utils as bass_utils

    global LAST_EXEC_NS
    c0 = _estimate_c0(logits, transitions)
    wF, wB, elF, elB, x0, z0 = _prep_device_inputs(logits, transitions, lens, c0)
    nc = _build_bass()
    in_maps = []
    for c in range(NCORES):
        in_maps.append({
            "wf": wF, "wb": wB,
            "elf": np.ascontiguousarray(elF[c]),
            "elb": np.ascontiguousarray(elB[c]),
            "x0": x0, "z0": z0,
        })
    res = bass_utils.run_bass_kernel_spmd(nc, in_maps, core_ids=list(range(NCORES)))
    LAST_EXEC_NS = getattr(res, "exec_time_ns", None)

    part = np.empty(B, np.float64)
    for c in range(NCORES):
        xf = res.results[c]["xout"].astype(np.float64)   # [128, 64]
        zf = res.results[c]["zout"].astype(np.float64)   # [128, 64]
        # final z: block A (ex 0..63) on partitions 64..127, block B on 0..63
        dotA = (xf[0:64, :] * zf[64:128, :]).sum(0)      # ex 0..63
        dotB = (xf[64:128, :] * zf[0:64, :]).sum(0)      # ex 64..127
        dot = np.concatenate([dotA, dotB])
        part[c * BC : (c + 1) * BC] = np.log(dot)
    return part + c0 * lens.astype(np.float64)


def _partition_cpu(logits, transitions, lens):
    """Exact f64 fallback (slow, emergencies only)."""
    lg = logits.astype(np.float64)
    tr = transitions.astype(np.float64)
    alpha = np.full((B, T), NEG)
    alpha[:, START] = 0.0
    for t in range(L):
        mat = tr[None] + alpha[:, None, :] + lg[:, t, :, None]
        mx = mat.max(2, keepdims=True)
        an = np.log(np.exp(mat - mx).sum(2)) + mx[:, :, 0]
        alpha = np.where((t < lens)[:, None], an, alpha)
    v = alpha + tr[STOP][None, :]
    mx = v.max(1, keepdims=True)
    return np.log(np.exp(v - mx).sum(1)) + mx[:, 0]


def kernel(**inputs):
    logits = np.asarray(inputs["logits"], np.float32)
    transitions = np.asarray(inputs["transitions"], np.float32)
    labels = np.asarray(inputs["labels"]).astype(np.int64)
    lens = np.asarray(inputs["lens"]).astype(np.int64)

    try:
        partition = _partition_device(logits, transitions, lens)
        if not np.all(np.isfinite(partition)):
            raise FloatingPointError("non-finite partition from device")
    except Exception:
        partition = _partition_cpu(logits, transitions, lens)

    tr = transitions.astype(np.float64)
    labels_ext = np.concatenate([
        np.full((B, 1), START, np.int64), labels,
        np.full((B, 1), STOP, np.int64)], 1)
    keep = np.arange(L + 2)[None, :] < (lens + 1)[:, None]
    labels_ext = np.where(keep, labels_ext, STOP)
    trn = tr[labels_ext[:, 1:], labels_ext[:, :-1]]
    tmask = (np.arange(L + 1)[None, :] < (lens + 1)[:, None]).astype(np.float64)
    trans_score = (trn * tmask).sum(1)

    em = np.take_along_axis(
        logits.astype(np.float64), labels[:, :, None], axis=2)[:, :, 0]
    emask = (np.arange(L)[None, :] < lens[:, None]).astype(np.float64)
    emission = (em * emask).sum(1)

    loss = (partition - emission - trans_score).sum() / B
    return np.asarray(loss, dtype=np.float32)
